# revision 28
# baseline (speedup 1.0000x reference)
"""Multi-head attention forward for Trainium2, 8 NeuronCores.

Problem: B=4, S=2048, D=1024, H=16 heads (dk=64), fp32 reference:
  q/k/v = x @ W{q,k,v}^T + b ; heads split; softmax(q k^T / 8) v ; out @ Wo^T + bo

Sharding: 8 cores = 4 batches x 2 head-groups (8 heads each), Megatron-style.
Host sums the two partial output projections per batch and adds the
bias row (Wo_b + sum_g Wv_b[g] @ Wo_w[:,g].T — the V-bias contributes
exactly bv to the normalized attention output, so it folds into a
constant output row).

Per-core kernel (all inputs fp16, PSUM accumulation fp32):
  V: V[s,d] = x@WvT (no bias), fp16 [k-part, feature] tiles
  A: QT/KT[dk, s] = (x@W^T)^T + bias per head-pair (128 feature rows)
  B: per head-pair, per 512-query step, per 128-key tile:
     S^T[k,q] pair of K=64 row-packed matmuls -> one [128,1024] f32 PSUM
     P = exp(S^T/8) one ACT instruction -> fp16 SBUF (per-kti tiles);
     scores for kti+1 are emitted ahead of PV(kti) (PE-queue pipelining),
     and injected A/C work is drained as 2-matmul chunks from a FIFO
     OT += V^T P (two M=64 col-packed fp16 matmuls into 1-bank PSUM)
     den += ones64 @ P (two M=64 col-packed matmuls -> den REPLICATED
       across the 64 dk rows of each head -> no cross-partition fixup)
     step end: rt = recip(den), OT_norm = OT * rt (one DVE mul)
  C: y_partial = OT_norm^T @ WoT (fp16 matmuls, no bias)
Phase A for pair p+1 is interleaved into phase B of pair p so the
tensor engine fills the slack under the ACT-bound softmax pipeline.
"""

import sys

sys.path.insert(0, "/opt/trn_rl_repo")

import numpy as np

import concourse.bass as bass  # noqa: F401
import concourse.mybir as mybir
import concourse.tile as tile
from concourse import bacc, bass_utils

B, S, D, H = 4, 2048, 1024, 16
DK = D // H          # 64
G = 2                # head groups (tensor-parallel factor)
DL = D // G          # 512 local features per core
NPAIR = DL // 128    # 4 head-pairs per core
EC = D // 128        # 8 contraction chunks for projections
ST = S // 128        # 16 s-tiles
KT = S // 128        # 16 key tiles
NQ = S // 512        # 4 query steps of 512

F32 = mybir.dt.float32
F16 = mybir.dt.float16

_CACHED = {}


def _build_nc(loop_n=1):
    nc = bacc.Bacc(None, target_bir_lowering=False)

    xT = nc.dram_tensor("xT", [D, S], F16, kind="ExternalInput")
    wqT = nc.dram_tensor("wqT", [D, DL], F16, kind="ExternalInput")
    wkT = nc.dram_tensor("wkT", [D, DL], F16, kind="ExternalInput")
    wvT = nc.dram_tensor("wvT", [D, DL], F16, kind="ExternalInput")
    woT = nc.dram_tensor("woT", [DL, D], F16, kind="ExternalInput")
    bq = nc.dram_tensor("bq", [DL], F32, kind="ExternalInput")
    bk = nc.dram_tensor("bk", [DL], F32, kind="ExternalInput")
    y = nc.dram_tensor("y", [S, D], F16, kind="ExternalOutput")

    with tile.TileContext(nc) as tc:
      for _rep in range(loop_n):
        with (
            tc.tile_pool(name="main", bufs=1) as pmain,
            tc.tile_pool(name="qkt", bufs=2) as pqkt,
            tc.tile_pool(name="ptile", bufs=6) as ppt,
            tc.tile_pool(name="pacc", bufs=2) as ppc,
            tc.tile_pool(name="rtile", bufs=2) as prt,
            tc.tile_pool(name="ytile", bufs=3) as pyt,
        ):
            # persistent tiles
            vt = pmain.tile([128, ST, DL], F16, tag="vt")
            ot = pmain.tile([128, NPAIR, S], F16, tag="ot")
            wot = pmain.tile([128, NPAIR, D], F16, tag="wot")
            ones64 = pmain.tile([128, 64], F16, tag="ones64")
            bqt = pmain.tile([128, NPAIR], F32, tag="bqt")
            bkt = pmain.tile([128, NPAIR], F32, tag="bkt")

            nc.vector.memset(ones64[:], 1.0)
            nc.sync.dma_start(bqt[:], bq.ap().rearrange("(p d) -> d p", d=128))
            nc.sync.dma_start(bkt[:], bk.ap().rearrange("(p d) -> d p", d=128))

            with (
                tc.tile_pool(name="xw", bufs=1) as pxw,
                tc.tile_pool(name="wqk", bufs=2) as pwqk,
                tc.tile_pool(name="psSC", bufs=3, space="PSUM") as psSC,
                tc.tile_pool(name="psOT", bufs=1, space="PSUM") as psOT,
                tc.tile_pool(name="psVA", bufs=1, space="PSUM") as psVA,
            ):
                xt = pxw.tile([128, EC, S], F16, tag="xt")
                wvt = pxw.tile([128, EC, DL], F16, tag="wvt")
                for ec in range(EC):
                    nc.sync.dma_start(xt[:, ec], xT.ap()[ec * 128:(ec + 1) * 128, :])
                    nc.sync.dma_start(wvt[:, ec], wvT.ap()[ec * 128:(ec + 1) * 128, :])

                def v_group(st):
                    # V = x @ WvT (bias folded to host), fp16 [k, d] layout
                    vps = psVA.tile([128, DL], F32, tag="g")
                    for ec in range(EC):
                        nc.tensor.matmul(
                            vps[:], xt[:, ec, st * 128:(st + 1) * 128], wvt[:, ec],
                            start=(ec == 0), stop=(ec == EC - 1))
                    nc.vector.tensor_copy(vt[:, st], vps[:])

                qts = [None] * NPAIR
                kts = [None] * NPAIR
                wps = [None] * NPAIR

                def load_wqk(p):
                    wqp = pwqk.tile([128, EC, 128], F16, tag="wqp")
                    wkp = pwqk.tile([128, EC, 128], F16, tag="wkp")
                    for ec in range(EC):
                        nc.sync.dma_start(
                            wqp[:, ec],
                            wqT.ap()[ec * 128:(ec + 1) * 128, p * 128:(p + 1) * 128])
                        nc.sync.dma_start(
                            wkp[:, ec],
                            wkT.ap()[ec * 128:(ec + 1) * 128, p * 128:(p + 1) * 128])
                    qt = pqkt.tile([128, S], F16, tag="qt")
                    kt = pqkt.tile([128, S], F16, tag="kt")
                    qts[p], kts[p], wps[p] = qt, kt, (wqp, wkp)

                def a_group(p, idx):
                    # one of 8 projection groups for pair p: q/k x 4 q-chunks
                    wqp, wkp = wps[p]
                    dst, wp, bias = ((qts[p], wqp, bqt) if idx < 4
                                     else (kts[p], wkp, bkt))
                    qc = idx % 4
                    ps = psVA.tile([128, 512], F32, tag="g")
                    for ec in range(EC):
                        nc.tensor.matmul(
                            ps[:], wp[:, ec], xt[:, ec, qc * 512:(qc + 1) * 512],
                            start=(ec == 0), stop=(ec == EC - 1))
                    nc.vector.tensor_scalar_add(
                        dst[:, qc * 512:(qc + 1) * 512], ps[:], bias[:, p:p + 1])

                work = []  # FIFO of ~2-matmul emission chunks

                def queue_a_group(p, idx):
                    # a_group split into 4 chunks so the PE bursts stay under
                    # the one-tile scores prefetch slack (no ACT bubbles)
                    wqp, wkp = wps[p]
                    dst, wp, bias = ((qts[p], wqp, bqt) if idx < 4
                                     else (kts[p], wkp, bkt))
                    qc = idx % 4
                    cell = {}

                    def chunk(e0):
                        def emit():
                            if e0 == 0:
                                aps = psVA.tile([128, 512], F32, tag="g")
                                cell["ps"] = aps
                            ps = cell["ps"]
                            for ec in (e0, e0 + 1):
                                nc.tensor.matmul(
                                    ps[:], wp[:, ec],
                                    xt[:, ec, qc * 512:(qc + 1) * 512],
                                    start=(ec == 0), stop=(ec == EC - 1))
                            if e0 == EC - 2:
                                nc.vector.tensor_scalar_add(
                                    dst[:, qc * 512:(qc + 1) * 512], ps[:],
                                    bias[:, p:p + 1])
                        return emit

                    work.extend(chunk(e0) for e0 in range(0, EC, 2))

                def queue_c_group(st, e2):
                    ss = slice(st * 128, (st + 1) * 128)
                    es = slice(e2 * 512, (e2 + 1) * 512)
                    cell = {}

                    def chunk(d0):
                        def emit():
                            if d0 == 0:
                                cps = psVA.tile([128, 512], F32, tag="g")
                                cell["ps"] = cps
                            yps = cell["ps"]
                            for dc in (d0, d0 + 1):
                                nc.tensor.matmul(
                                    yps[:], ot[:, dc, ss], wot[:, dc, es],
                                    start=(dc == 0), stop=(dc == NPAIR - 1))
                            if d0 == NPAIR - 2:
                                yt = pyt.tile([128, 512], F16, tag="yt")
                                nc.vector.tensor_copy(yt[:], yps[:])
                                nc.sync.dma_start(y.ap()[ss, es], yt[:])
                        return emit

                    work.extend(chunk(d0) for d0 in range(0, NPAIR, 2))

                def c_group(st, e2):
                    # one output-projection group; emitted late enough that
                    # ot rows [st*128, st*128+128) are final for all pairs
                    ss = slice(st * 128, (st + 1) * 128)
                    es = slice(e2 * 512, (e2 + 1) * 512)
                    yps = psVA.tile([128, 512], F32, tag="g")
                    for dc in range(NPAIR):
                        nc.tensor.matmul(
                            yps[:], ot[:, dc, ss], wot[:, dc, es],
                            start=(dc == 0), stop=(dc == NPAIR - 1))
                    yt = pyt.tile([128, 512], F16, tag="yt")
                    nc.vector.tensor_copy(yt[:], yps[:])
                    nc.sync.dma_start(y.ap()[ss, es], yt[:])

                load_wqk(0)
                for idx in range(8):
                    a_group(0, idx)

                carry = {"sc": None}
                for p in range(NPAIR):
                    if p + 1 < NPAIR:
                        load_wqk(p + 1)
                        if p >= 1:
                            for idx in range(8):
                                queue_a_group(p + 1, idx)
                    if p == NPAIR - 1:
                        for dc in range(NPAIR):
                            nc.sync.dma_start(
                                wot[:, dc], woT.ap()[dc * 128:(dc + 1) * 128, :])
                    qt, kt = qts[p], kts[p]
                    for qq in range(NQ):
                        q0 = qq * 512
                        qs = slice(q0, q0 + 512)
                        if p == 0 and qq == 1:
                            for idx in range(8):
                                queue_a_group(1, idx)
                        if p == NPAIR - 1 and qq == 2:
                            for st in range(4):
                                for e2 in range(2):
                                    queue_c_group(st, e2)
                        otp = psOT.tile([128, 512], F32, tag="otp")
                        pacc = ppc.tile([128, 1024], F16, tag="pacc")

                        def scores(kti, qt_=None, kt_=None, qs_=None):
                            # PE stage 1, emitted one step AHEAD of PV(kti-1)
                            # so the in-order PE queue never idles behind the
                            # exp(kti-1) wait.
                            qt_, kt_ = qt_ if qt_ is not None else qt, \
                                kt_ if kt_ is not None else kt
                            qs_ = qs_ if qs_ is not None else qs
                            ks = slice(kti * 128, (kti + 1) * 128)
                            sc = psSC.tile([128, 1024], F32, tag="sc")
                            nc.tensor.matmul(
                                sc[:, 0:512], kt_[0:64, ks], qt_[0:64, qs_],
                                start=True, stop=True, tile_position=(0, 0))
                            nc.tensor.matmul(
                                sc[:, 512:1024], kt_[64:128, ks],
                                qt_[64:128, qs_],
                                start=True, stop=True, tile_position=(64, 0))
                            return sc

                        if p == 0 and qq == 0:
                            v_group(0)
                        if carry["sc"] is not None:
                            sc_next = carry["sc"]
                            carry["sc"] = None
                        else:
                            sc_next = scores(0)
                        for kti in range(KT):
                            if p == 0 and qq == 0 and kti + 1 < KT:
                                v_group(kti + 1)  # JIT-build V under B(0,qq0)
                            p2 = ppt.tile([128, 1024], F16, tag="p2")
                            c0 = 0
                            sc = sc_next
                            if kti + 1 < KT:
                                sc_next = scores(kti + 1)
                            nc.scalar.activation(
                                p2[:, c0:c0 + 1024], sc[:],
                                mybir.ActivationFunctionType.Exp, scale=0.125)
                            first, last = kti == 0, kti == KT - 1
                            nc.tensor.matmul(
                                otp[0:64, :],
                                vt[:, kti, p * 128:p * 128 + 64],
                                p2[:, c0:c0 + 512],
                                start=first, stop=last, tile_position=(0, 0))
                            nc.tensor.matmul(
                                otp[64:128, :],
                                vt[:, kti, p * 128 + 64:(p + 1) * 128],
                                p2[:, c0 + 512:c0 + 1024],
                                start=first, stop=last, tile_position=(0, 64))
                            if first:
                                nc.vector.tensor_copy(
                                    pacc[:], p2[:, c0:c0 + 1024])
                            else:
                                nc.vector.tensor_add(
                                    pacc[:], pacc[:], p2[:, c0:c0 + 1024])
                            if work:
                                work.pop(0)()
                        # prefetch the next step's first scores ahead of the
                        # den matmuls (which wait on the last pacc add), so
                        # the PE queue keeps feeding ACT across the boundary
                        if qq + 1 < NQ:
                            carry["sc"] = scores(0, qs_=slice(q0 + 512,
                                                              q0 + 1024))
                        elif p + 1 < NPAIR:
                            carry["sc"] = scores(0, qt_=qts[p + 1],
                                                 kt_=kts[p + 1],
                                                 qs_=slice(0, 512))
                        dnp = psVA.tile([128, 512], F32, tag="g")
                        nc.tensor.matmul(
                            dnp[0:64, :], ones64[:], pacc[:, 0:512],
                            start=True, stop=True, tile_position=(0, 0))
                        nc.tensor.matmul(
                            dnp[64:128, :], ones64[:], pacc[:, 512:1024],
                            start=True, stop=True, tile_position=(0, 64))
                        rt = prt.tile([128, 512], F32, tag="rt")
                        nc.vector.reciprocal(rt[:], dnp[:])
                        nc.vector.tensor_mul(ot[:, p, qs], otp[:], rt[:])

                while work:
                    work.pop(0)()
                # remaining output-projection groups (st 4..15)
                for st in range(4, ST):
                    for e2 in range(2):
                        c_group(st, e2)

    nc.compile()
    return nc


def _get_nc(loop_n=1):
    key = f"nc{loop_n}"
    if key not in _CACHED:
        _CACHED[key] = _build_nc(loop_n)
    return _CACHED[key]


def make_in_maps(encoder_input, Wq_w, Wq_b, Wk_w, Wk_b, Wv_w, Wo_w):
    """Per-core input maps for the 8-core SPMD kernel (fp16 inputs)."""
    enc = np.asarray(encoder_input, dtype=np.float32)
    woT_f16, wqT_f16, wkT_f16, wvT_f16 = {}, {}, {}, {}
    in_maps = []
    for core in range(8):
        b, g = divmod(core, G)
        gs = slice(g * DL, (g + 1) * DL)
        if g not in woT_f16:
            woT_f16[g] = np.ascontiguousarray(
                np.asarray(Wo_w)[:, gs].T).astype(np.float16)
            wqT_f16[g] = np.ascontiguousarray(
                np.asarray(Wq_w)[gs, :].T).astype(np.float16)
            wkT_f16[g] = np.ascontiguousarray(
                np.asarray(Wk_w)[gs, :].T).astype(np.float16)
            wvT_f16[g] = np.ascontiguousarray(
                np.asarray(Wv_w)[gs, :].T).astype(np.float16)
        in_maps.append({
            "xT": np.ascontiguousarray(enc[b].T).astype(np.float16),
            "wqT": wqT_f16[g],
            "wkT": wkT_f16[g],
            "wvT": wvT_f16[g],
            "woT": woT_f16[g],
            "bq": np.ascontiguousarray(np.asarray(Wq_b, np.float32)[gs]),
            "bk": np.ascontiguousarray(np.asarray(Wk_b, np.float32)[gs]),
        })
    return in_maps


def host_bias_row(Wv_b, Wo_w, Wo_b):
    """Constant output row: Wo_b plus the folded V-bias contribution."""
    Wv_b = np.asarray(Wv_b, np.float32)
    Wo_w = np.asarray(Wo_w, np.float32)
    c = np.asarray(Wo_b, np.float32).copy()
    for g in range(G):
        gs = slice(g * DL, (g + 1) * DL)
        c = c + Wv_b[gs] @ Wo_w[:, gs].T
    return c


def _get_runner():
    """Build the 8-core SPMD executable once and cache it, so repeated
    kernel() calls skip jax re-tracing and NEFF compilation."""
    if "runner" in _CACHED:
        return _CACHED["runner"]

    import jax
    from jax.sharding import Mesh, NamedSharding, PartitionSpec
    from jax.experimental.shard_map import shard_map
    from concourse import bass2jax
    from concourse.bass2jax import _bass_exec_p, install_neuronx_cc_hook

    nc = _get_nc()
    install_neuronx_cc_hook()
    partition_name = nc.partition_id_tensor.name if nc.partition_id_tensor else None
    in_names, out_names, out_avals, zero_outs = [], [], [], []
    for alloc in nc.m.functions[0].allocations:
        if not isinstance(alloc, mybir.MemoryLocationSet):
            continue
        name = alloc.memorylocations[0].name
        if alloc.kind == "ExternalInput":
            if name != partition_name:
                in_names.append(name)
        elif alloc.kind == "ExternalOutput":
            out_names.append(name)
            shape = tuple(alloc.tensor_shape)
            dtype = mybir.dt.np(alloc.dtype)
            out_avals.append(jax.core.ShapedArray(shape, dtype))
            zero_outs.append(np.zeros(shape, dtype))
    n_params, n_outs = len(in_names), len(out_avals)
    all_names = in_names + out_names + ([partition_name] if partition_name else [])

    def _body(*args):
        operands = list(args)
        if partition_name is not None:
            operands.append(bass2jax.partition_id_tensor())
        outs = _bass_exec_p.bind(
            *operands,
            out_avals=tuple(out_avals),
            in_names=tuple(all_names),
            out_names=tuple(out_names),
            lowering_input_output_aliases=(),
            sim_require_finite=True,
            sim_require_nnan=True,
            nc=nc,
        )
        return tuple(outs)

    devices = jax.devices()[:8]
    mesh = Mesh(np.asarray(devices), ("core",))
    f = jax.jit(
        shard_map(
            _body, mesh=mesh,
            in_specs=(PartitionSpec("core"),) * (n_params + n_outs),
            out_specs=(PartitionSpec("core"),) * n_outs,
            check_rep=False,
        ),
        donate_argnums=tuple(range(n_params, n_params + n_outs)),
        keep_unused=True,
    )
    shard = NamedSharding(mesh, PartitionSpec("core"))
    state = {
        "f": f, "in_names": in_names, "out_names": out_names,
        "zero_outs": zero_outs, "shard": shard, "jax": jax, "last_outs": None,
    }
    _CACHED["runner"] = state
    return state


def kernel(encoder_input, attention_mask, Wq_w, Wq_b, Wk_w, Wk_b, Wv_w, Wv_b,
           Wo_w, Wo_b):
    del attention_mask  # dead input in the reference forward

    r = _get_runner()
    jax = r["jax"]

    in_maps = make_in_maps(encoder_input, Wq_w, Wq_b, Wk_w, Wk_b, Wv_w, Wo_w)
    c_row = host_bias_row(Wv_b, Wo_w, Wo_b)

    concat_in = [
        jax.device_put(
            np.concatenate([in_maps[c][n] for c in range(8)], axis=0), r["shard"])
        for n in r["in_names"]
    ]
    outs = r["last_outs"]
    if outs is None:
        outs = [
            jax.device_put(
                np.zeros((8 * z.shape[0], *z.shape[1:]), z.dtype), r["shard"])
            for z in r["zero_outs"]
        ]
    outs = r["f"](*concat_in, *outs)
    np_outs = [np.asarray(o) for o in outs]
    # keep the returned device buffers to donate on the next call
    r["last_outs"] = list(outs)

    per_core = {}
    for i, nme in enumerate(r["out_names"]):
        full = np_outs[i].reshape(8, -1, *np_outs[i].shape[1:])
        per_core[nme] = full

    yv = per_core["y"]
    out = np.empty((B, S, D), dtype=np.float32)
    for b in range(B):
        out[b] = (yv[G * b].astype(np.float32)
                  + yv[G * b + 1].astype(np.float32) + c_row)
    return out


# revision 30
# speedup vs baseline: 1.1514x; 1.1514x over previous
"""Multi-head attention forward for Trainium2, 8 NeuronCores.

Problem: B=4, S=2048, D=1024, H=16 heads (dk=64), fp32 reference:
  q/k/v = x @ W{q,k,v}^T + b ; heads split; softmax(q k^T / 8) v ; out @ Wo^T + bo

Sharding: 8 cores = 4 batches x 2 head-groups (8 heads each), Megatron-style.
Host sums the two partial output projections per batch and adds the
bias row (Wo_b + sum_g Wv_b[g] @ Wo_w[:,g].T — the V-bias contributes
exactly bv to the normalized attention output, so it folds into a
constant output row).

Per-core kernel (all inputs fp16, PSUM accumulation fp32):
  V: V[s,d] = x@WvT (no bias), fp16 [k-part, feature] tiles
  A: QT/KT[dk, s] = (x@W^T)^T + bias per head-pair (128 feature rows)
  B: per head-pair, per 512-query step, per 128-key tile:
     S^T[k,q] pair of K=64 row-packed matmuls -> one [128,1024] f32 PSUM
     P = exp(S^T/8) one ACT instruction -> fp16 SBUF (per-kti tiles);
     scores for kti+1 are emitted ahead of PV(kti) (PE-queue pipelining),
     and injected A/C work is drained as 2-matmul chunks from a FIFO
     OT += V^T P (two M=64 col-packed fp16 matmuls into 1-bank PSUM)
     den += ones64 @ P (two M=64 col-packed matmuls -> den REPLICATED
       across the 64 dk rows of each head -> no cross-partition fixup)
     step end: rt = recip(den), OT_norm = OT * rt (one DVE mul)
  C: y_partial = OT_norm^T @ WoT (fp16 matmuls, no bias)
Phase A for pair p+1 is interleaved into phase B of pair p so the
tensor engine fills the slack under the ACT-bound softmax pipeline.
"""

import sys

sys.path.insert(0, "/opt/trn_rl_repo")

import numpy as np

import concourse.bass as bass  # noqa: F401
import concourse.mybir as mybir
import concourse.tile as tile
from concourse import bacc, bass_utils

B, S, D, H = 4, 2048, 1024, 16
DK = D // H          # 64
G = 2                # head groups (tensor-parallel factor)
DL = D // G          # 512 local features per core
NPAIR = DL // 128    # 4 head-pairs per core
EC = D // 128        # 8 contraction chunks for projections
ST = S // 128        # 16 s-tiles
KT = S // 128        # 16 key tiles
NQ = S // 512        # 4 query steps of 512

F32 = mybir.dt.float32
F16 = mybir.dt.float16

_CACHED = {}


def _build_nc(loop_n=1):
    nc = bacc.Bacc(None, target_bir_lowering=False)

    xT = nc.dram_tensor("xT", [D, S], F16, kind="ExternalInput")
    wqT = nc.dram_tensor("wqT", [D, DL], F16, kind="ExternalInput")
    wkT = nc.dram_tensor("wkT", [D, DL], F16, kind="ExternalInput")
    wvT = nc.dram_tensor("wvT", [D, DL], F16, kind="ExternalInput")
    woT = nc.dram_tensor("woT", [DL, D], F16, kind="ExternalInput")
    bq = nc.dram_tensor("bq", [DL], F32, kind="ExternalInput")
    bk = nc.dram_tensor("bk", [DL], F32, kind="ExternalInput")
    y = nc.dram_tensor("y", [S, D], F16, kind="ExternalOutput")

    with tile.TileContext(nc) as tc:
      for _rep in range(loop_n):
        with (
            tc.tile_pool(name="main", bufs=1) as pmain,
            tc.tile_pool(name="qkt", bufs=2) as pqkt,
            tc.tile_pool(name="ptile", bufs=6) as ppt,
            tc.tile_pool(name="pacc", bufs=2) as ppc,
            tc.tile_pool(name="rtile", bufs=2) as prt,
            tc.tile_pool(name="ytile", bufs=3) as pyt,
        ):
            # persistent tiles
            vt = pmain.tile([128, ST, DL], F16, tag="vt")
            ot = pmain.tile([128, NPAIR, S], F16, tag="ot")
            wot = pmain.tile([128, NPAIR, D], F16, tag="wot")
            ones64 = pmain.tile([128, 64], F16, tag="ones64")
            bqt = pmain.tile([128, NPAIR], F32, tag="bqt")
            bkt = pmain.tile([128, NPAIR], F32, tag="bkt")

            nc.vector.memset(ones64[:], 1.0)
            nc.sync.dma_start(bqt[:], bq.ap().rearrange("(p d) -> d p", d=128))
            nc.sync.dma_start(bkt[:], bk.ap().rearrange("(p d) -> d p", d=128))

            with (
                tc.tile_pool(name="xw", bufs=1) as pxw,
                tc.tile_pool(name="wqk", bufs=2) as pwqk,
                tc.tile_pool(name="psSC", bufs=2, space="PSUM") as psSC,
                tc.tile_pool(name="psOT", bufs=2, space="PSUM") as psOT,
                tc.tile_pool(name="psVA", bufs=2, space="PSUM") as psVA,
            ):
                xt = pxw.tile([128, EC, S], F16, tag="xt")
                wvt = pxw.tile([128, EC, DL], F16, tag="wvt")
                for ec in range(EC):
                    nc.sync.dma_start(xt[:, ec], xT.ap()[ec * 128:(ec + 1) * 128, :])
                    nc.sync.dma_start(wvt[:, ec], wvT.ap()[ec * 128:(ec + 1) * 128, :])

                def v_group(st):
                    # V = x @ WvT (bias folded to host), fp16 [k, d] layout
                    vps = psVA.tile([128, DL], F32, tag="g")
                    for ec in range(EC):
                        nc.tensor.matmul(
                            vps[:], xt[:, ec, st * 128:(st + 1) * 128], wvt[:, ec],
                            start=(ec == 0), stop=(ec == EC - 1))
                    nc.vector.tensor_copy(vt[:, st], vps[:])

                qts = [None] * NPAIR
                kts = [None] * NPAIR
                wps = [None] * NPAIR

                def load_wqk(p):
                    wqp = pwqk.tile([128, EC, 128], F16, tag="wqp")
                    wkp = pwqk.tile([128, EC, 128], F16, tag="wkp")
                    for ec in range(EC):
                        nc.sync.dma_start(
                            wqp[:, ec],
                            wqT.ap()[ec * 128:(ec + 1) * 128, p * 128:(p + 1) * 128])
                        nc.sync.dma_start(
                            wkp[:, ec],
                            wkT.ap()[ec * 128:(ec + 1) * 128, p * 128:(p + 1) * 128])
                    qt = pqkt.tile([128, S], F16, tag="qt")
                    kt = pqkt.tile([128, S], F16, tag="kt")
                    qts[p], kts[p], wps[p] = qt, kt, (wqp, wkp)

                def a_group(p, idx):
                    # one of 8 projection groups for pair p: q/k x 4 q-chunks
                    wqp, wkp = wps[p]
                    dst, wp, bias = ((qts[p], wqp, bqt) if idx < 4
                                     else (kts[p], wkp, bkt))
                    qc = idx % 4
                    ps = psVA.tile([128, 512], F32, tag="g")
                    for ec in range(EC):
                        nc.tensor.matmul(
                            ps[:], wp[:, ec], xt[:, ec, qc * 512:(qc + 1) * 512],
                            start=(ec == 0), stop=(ec == EC - 1))
                    nc.vector.tensor_scalar_add(
                        dst[:, qc * 512:(qc + 1) * 512], ps[:], bias[:, p:p + 1])

                work = []  # FIFO of ~2-matmul emission chunks

                def queue_a_group(p, idx):
                    # a_group split into 4 chunks so the PE bursts stay under
                    # the one-tile scores prefetch slack (no ACT bubbles)
                    wqp, wkp = wps[p]
                    dst, wp, bias = ((qts[p], wqp, bqt) if idx < 4
                                     else (kts[p], wkp, bkt))
                    qc = idx % 4
                    cell = {}

                    def chunk(e0):
                        def emit():
                            if e0 == 0:
                                aps = psVA.tile([128, 512], F32, tag="g")
                                cell["ps"] = aps
                            ps = cell["ps"]
                            for ec in (e0, e0 + 1):
                                nc.tensor.matmul(
                                    ps[:], wp[:, ec],
                                    xt[:, ec, qc * 512:(qc + 1) * 512],
                                    start=(ec == 0), stop=(ec == EC - 1))
                            if e0 == EC - 2:
                                nc.vector.tensor_scalar_add(
                                    dst[:, qc * 512:(qc + 1) * 512], ps[:],
                                    bias[:, p:p + 1])
                        return emit

                    work.extend(chunk(e0) for e0 in range(0, EC, 2))

                def queue_c_group(st, e2):
                    ss = slice(st * 128, (st + 1) * 128)
                    es = slice(e2 * 512, (e2 + 1) * 512)
                    cell = {}

                    def chunk(d0):
                        def emit():
                            if d0 == 0:
                                cps = psVA.tile([128, 512], F32, tag="g")
                                cell["ps"] = cps
                            yps = cell["ps"]
                            for dc in (d0, d0 + 1):
                                nc.tensor.matmul(
                                    yps[:], ot[:, dc, ss], wot[:, dc, es],
                                    start=(dc == 0), stop=(dc == NPAIR - 1))
                            if d0 == NPAIR - 2:
                                yt = pyt.tile([128, 512], F16, tag="yt")
                                nc.vector.tensor_copy(yt[:], yps[:])
                                nc.sync.dma_start(y.ap()[ss, es], yt[:])
                        return emit

                    work.extend(chunk(d0) for d0 in range(0, NPAIR, 2))

                def c_group(st, e2):
                    # one output-projection group; emitted late enough that
                    # ot rows [st*128, st*128+128) are final for all pairs
                    ss = slice(st * 128, (st + 1) * 128)
                    es = slice(e2 * 512, (e2 + 1) * 512)
                    yps = psVA.tile([128, 512], F32, tag="g")
                    for dc in range(NPAIR):
                        nc.tensor.matmul(
                            yps[:], ot[:, dc, ss], wot[:, dc, es],
                            start=(dc == 0), stop=(dc == NPAIR - 1))
                    yt = pyt.tile([128, 512], F16, tag="yt")
                    nc.vector.tensor_copy(yt[:], yps[:])
                    nc.sync.dma_start(y.ap()[ss, es], yt[:])

                load_wqk(0)
                for idx in range(8):
                    a_group(0, idx)

                carry = {"sc": None}
                for p in range(NPAIR):
                    if p + 1 < NPAIR:
                        load_wqk(p + 1)
                        if p >= 1:
                            for idx in range(8):
                                queue_a_group(p + 1, idx)
                    if p == NPAIR - 1:
                        for dc in range(NPAIR):
                            nc.sync.dma_start(
                                wot[:, dc], woT.ap()[dc * 128:(dc + 1) * 128, :])
                    qt, kt = qts[p], kts[p]
                    for qq in range(NQ):
                        q0 = qq * 512
                        qs = slice(q0, q0 + 512)
                        if p == 0 and qq == 1:
                            for idx in range(8):
                                queue_a_group(1, idx)
                        if p == NPAIR - 1 and qq == 2:
                            for st in range(8):
                                for e2 in range(2):
                                    queue_c_group(st, e2)
                        otp = psOT.tile([128, 512], F32, tag="otp")
                        pacc = ppc.tile([128, 1024], F16, tag="pacc")

                        def scores(kti, qt_=None, kt_=None, qs_=None):
                            # PE stage 1, emitted one step AHEAD of PV(kti-1)
                            # so the in-order PE queue never idles behind the
                            # exp(kti-1) wait.
                            qt_, kt_ = qt_ if qt_ is not None else qt, \
                                kt_ if kt_ is not None else kt
                            qs_ = qs_ if qs_ is not None else qs
                            ks = slice(kti * 128, (kti + 1) * 128)
                            sc = psSC.tile([128, 1024], F32, tag="sc")
                            nc.tensor.matmul(
                                sc[:, 0:512], kt_[0:64, ks], qt_[0:64, qs_],
                                start=True, stop=True, tile_position=(0, 0))
                            nc.tensor.matmul(
                                sc[:, 512:1024], kt_[64:128, ks],
                                qt_[64:128, qs_],
                                start=True, stop=True, tile_position=(64, 0))
                            return sc

                        if p == 0 and qq == 0:
                            v_group(0)
                        if carry["sc"] is not None:
                            sc_next = carry["sc"]
                            carry["sc"] = None
                        else:
                            sc_next = scores(0)
                        for kti in range(KT):
                            if p == 0 and qq == 0 and kti + 1 < KT:
                                v_group(kti + 1)  # JIT-build V under B(0,qq0)
                            p2 = ppt.tile([128, 1024], F16, tag="p2")
                            c0 = 0
                            sc = sc_next
                            if kti + 1 < KT:
                                sc_next = scores(kti + 1)
                            nc.scalar.activation(
                                p2[:, c0:c0 + 1024], sc[:],
                                mybir.ActivationFunctionType.Exp, scale=0.125)
                            first, last = kti == 0, kti == KT - 1
                            nc.tensor.matmul(
                                otp[0:64, :],
                                vt[:, kti, p * 128:p * 128 + 64],
                                p2[:, c0:c0 + 512],
                                start=first, stop=last, tile_position=(0, 0))
                            nc.tensor.matmul(
                                otp[64:128, :],
                                vt[:, kti, p * 128 + 64:(p + 1) * 128],
                                p2[:, c0 + 512:c0 + 1024],
                                start=first, stop=last, tile_position=(0, 64))
                            if first:
                                nc.vector.tensor_copy(
                                    pacc[:], p2[:, c0:c0 + 1024])
                            else:
                                nc.vector.tensor_add(
                                    pacc[:], pacc[:], p2[:, c0:c0 + 1024])
                            if work:
                                work.pop(0)()
                        # prefetch the next step's first scores ahead of the
                        # den matmuls (which wait on the last pacc add), so
                        # the PE queue keeps feeding ACT across the boundary
                        if qq + 1 < NQ:
                            carry["sc"] = scores(0, qs_=slice(q0 + 512,
                                                              q0 + 1024))
                        elif p + 1 < NPAIR:
                            carry["sc"] = scores(0, qt_=qts[p + 1],
                                                 kt_=kts[p + 1],
                                                 qs_=slice(0, 512))
                        dnp = psVA.tile([128, 512], F32, tag="g")
                        nc.tensor.matmul(
                            dnp[0:64, :], ones64[:], pacc[:, 0:512],
                            start=True, stop=True, tile_position=(0, 0))
                        nc.tensor.matmul(
                            dnp[64:128, :], ones64[:], pacc[:, 512:1024],
                            start=True, stop=True, tile_position=(0, 64))
                        rt = prt.tile([128, 512], F32, tag="rt")
                        nc.vector.reciprocal(rt[:], dnp[:])
                        nc.vector.tensor_mul(ot[:, p, qs], otp[:], rt[:])

                while work:
                    work.pop(0)()
                # remaining output-projection groups (st 8..15)
                for st in range(8, ST):
                    for e2 in range(2):
                        c_group(st, e2)

    nc.compile()
    return nc


def _get_nc(loop_n=1):
    key = f"nc{loop_n}"
    if key not in _CACHED:
        _CACHED[key] = _build_nc(loop_n)
    return _CACHED[key]


def make_in_maps(encoder_input, Wq_w, Wq_b, Wk_w, Wk_b, Wv_w, Wo_w):
    """Per-core input maps for the 8-core SPMD kernel (fp16 inputs)."""
    enc = np.asarray(encoder_input, dtype=np.float32)
    woT_f16, wqT_f16, wkT_f16, wvT_f16 = {}, {}, {}, {}
    in_maps = []
    for core in range(8):
        b, g = divmod(core, G)
        gs = slice(g * DL, (g + 1) * DL)
        if g not in woT_f16:
            woT_f16[g] = np.ascontiguousarray(
                np.asarray(Wo_w)[:, gs].T).astype(np.float16)
            wqT_f16[g] = np.ascontiguousarray(
                np.asarray(Wq_w)[gs, :].T).astype(np.float16)
            wkT_f16[g] = np.ascontiguousarray(
                np.asarray(Wk_w)[gs, :].T).astype(np.float16)
            wvT_f16[g] = np.ascontiguousarray(
                np.asarray(Wv_w)[gs, :].T).astype(np.float16)
        in_maps.append({
            "xT": np.ascontiguousarray(enc[b].T).astype(np.float16),
            "wqT": wqT_f16[g],
            "wkT": wkT_f16[g],
            "wvT": wvT_f16[g],
            "woT": woT_f16[g],
            "bq": np.ascontiguousarray(np.asarray(Wq_b, np.float32)[gs]),
            "bk": np.ascontiguousarray(np.asarray(Wk_b, np.float32)[gs]),
        })
    return in_maps


def host_bias_row(Wv_b, Wo_w, Wo_b):
    """Constant output row: Wo_b plus the folded V-bias contribution."""
    Wv_b = np.asarray(Wv_b, np.float32)
    Wo_w = np.asarray(Wo_w, np.float32)
    c = np.asarray(Wo_b, np.float32).copy()
    for g in range(G):
        gs = slice(g * DL, (g + 1) * DL)
        c = c + Wv_b[gs] @ Wo_w[:, gs].T
    return c


def _get_runner():
    """Build the 8-core SPMD executable once and cache it, so repeated
    kernel() calls skip jax re-tracing and NEFF compilation."""
    if "runner" in _CACHED:
        return _CACHED["runner"]

    import jax
    from jax.sharding import Mesh, NamedSharding, PartitionSpec
    from jax.experimental.shard_map import shard_map
    from concourse import bass2jax
    from concourse.bass2jax import _bass_exec_p, install_neuronx_cc_hook

    nc = _get_nc()
    install_neuronx_cc_hook()
    partition_name = nc.partition_id_tensor.name if nc.partition_id_tensor else None
    in_names, out_names, out_avals, zero_outs = [], [], [], []
    for alloc in nc.m.functions[0].allocations:
        if not isinstance(alloc, mybir.MemoryLocationSet):
            continue
        name = alloc.memorylocations[0].name
        if alloc.kind == "ExternalInput":
            if name != partition_name:
                in_names.append(name)
        elif alloc.kind == "ExternalOutput":
            out_names.append(name)
            shape = tuple(alloc.tensor_shape)
            dtype = mybir.dt.np(alloc.dtype)
            out_avals.append(jax.core.ShapedArray(shape, dtype))
            zero_outs.append(np.zeros(shape, dtype))
    n_params, n_outs = len(in_names), len(out_avals)
    all_names = in_names + out_names + ([partition_name] if partition_name else [])

    def _body(*args):
        operands = list(args)
        if partition_name is not None:
            operands.append(bass2jax.partition_id_tensor())
        outs = _bass_exec_p.bind(
            *operands,
            out_avals=tuple(out_avals),
            in_names=tuple(all_names),
            out_names=tuple(out_names),
            lowering_input_output_aliases=(),
            sim_require_finite=True,
            sim_require_nnan=True,
            nc=nc,
        )
        return tuple(outs)

    devices = jax.devices()[:8]
    mesh = Mesh(np.asarray(devices), ("core",))
    f = jax.jit(
        shard_map(
            _body, mesh=mesh,
            in_specs=(PartitionSpec("core"),) * (n_params + n_outs),
            out_specs=(PartitionSpec("core"),) * n_outs,
            check_rep=False,
        ),
        donate_argnums=tuple(range(n_params, n_params + n_outs)),
        keep_unused=True,
    )
    shard = NamedSharding(mesh, PartitionSpec("core"))
    state = {
        "f": f, "in_names": in_names, "out_names": out_names,
        "zero_outs": zero_outs, "shard": shard, "jax": jax, "last_outs": None,
    }
    _CACHED["runner"] = state
    return state


def kernel(encoder_input, attention_mask, Wq_w, Wq_b, Wk_w, Wk_b, Wv_w, Wv_b,
           Wo_w, Wo_b):
    del attention_mask  # dead input in the reference forward

    r = _get_runner()
    jax = r["jax"]

    in_maps = make_in_maps(encoder_input, Wq_w, Wq_b, Wk_w, Wk_b, Wv_w, Wo_w)
    c_row = host_bias_row(Wv_b, Wo_w, Wo_b)

    concat_in = [
        jax.device_put(
            np.concatenate([in_maps[c][n] for c in range(8)], axis=0), r["shard"])
        for n in r["in_names"]
    ]
    outs = r["last_outs"]
    if outs is None:
        outs = [
            jax.device_put(
                np.zeros((8 * z.shape[0], *z.shape[1:]), z.dtype), r["shard"])
            for z in r["zero_outs"]
        ]
    outs = r["f"](*concat_in, *outs)
    np_outs = [np.asarray(o) for o in outs]
    # keep the returned device buffers to donate on the next call
    r["last_outs"] = list(outs)

    per_core = {}
    for i, nme in enumerate(r["out_names"]):
        full = np_outs[i].reshape(8, -1, *np_outs[i].shape[1:])
        per_core[nme] = full

    yv = per_core["y"]
    out = np.empty((B, S, D), dtype=np.float32)
    for b in range(B):
        out[b] = (yv[G * b].astype(np.float32)
                  + yv[G * b + 1].astype(np.float32) + c_row)
    return out
